# revision 2
# baseline (speedup 1.0000x reference)
"""BatchWhiten Trainium2 kernel (8-core SPMD, Bass/Tile).

y = x @ inv_sqrtm(max(0.1*running_covar + 0.9*(x^T x)/N, 1e-5))

Strategy (data-parallel over rows):
  - shard x row-wise across 8 cores
  - each core: S_c = x_c^T x_c via fp16 matmuls accumulated in fp32 PSUM
  - AllReduce the 512x512 partial (pre-scaled by 0.9/N)
  - replicated inverse-sqrt via coupled Newton-Schulz (fp16) + one fp32
    polish step (no SVD needed: spectrum is ~[0.95, 1.4])
  - each core whitens its shard: PE-transpose x tiles, y = x @ B in fp16

PSUM budget (8 banks): p2ps 4 (whole kernel) | covps 4 (phase 1 only)
then nsps 3 (after covps closes).
"""

import numpy as np

import concourse.bacc as bacc
import concourse.tile as tile
import concourse.mybir as mybir
from concourse import bass_utils

N_CORES = 8
D = 512
P = 128
MC = D // P              # 4 feature chunks of 128
N_TOTAL = 262144
SHARD = N_TOTAL // N_CORES
G = 8                    # row-tiles (128 rows each) per DMA chunk
MOMENTUM = 0.1
EPS = 1e-5
NS_ITERS = 6
C_SCALE = 2.0            # spectral normalizer for Newton-Schulz
INV_SQRT_C = 1.0 / np.sqrt(C_SCALE)

f32 = mybir.dt.float32
f16 = mybir.dt.float16


def _cols(mi):
    return slice(mi * P, (mi + 1) * P)


def build_program(shard=SHARD, n_total=N_TOTAL, ns_iters=NS_ITERS, g=G):
    """Build the SPMD Bass program. Returns compiled Bacc instance."""
    tpc = shard // P          # row-tiles per core
    nchunk = tpc // g
    assert nchunk * g == tpc

    nc = bacc.Bacc(
        "TRN2", target_bir_lowering=False, debug=False, num_devices=N_CORES
    )
    x_d = nc.dram_tensor("x", [shard, D], f32, kind="ExternalInput")
    rc_d = nc.dram_tensor("running_covar", [D, D], f32, kind="ExternalInput")
    eye15_d = nc.dram_tensor("eye15", [D, D], f16, kind="ExternalInput")
    id16_d = nc.dram_tensor("id128_16", [P, P], f16, kind="ExternalInput")
    id32_d = nc.dram_tensor("id128_32", [P, P], f32, kind="ExternalInput")
    eye16_d = nc.dram_tensor("eye16", [D, D], f16, kind="ExternalInput")
    y_d = nc.dram_tensor("y", [shard, D], f32, kind="ExternalOutput")

    # partition-major DRAM views: [p, tile_idx, feat]
    x_v = x_d.ap().rearrange("(n p) m -> p n m", p=P)
    y_v = y_d.ap().rearrange("(n p) m -> p n m", p=P)
    rc_v = rc_d.ap().rearrange("(t p) m -> p t m", p=P)
    e15_v = eye15_d.ap().rearrange("(t p) m -> p t m", p=P)
    e16_v = eye16_d.ap().rearrange("(t p) m -> p t m", p=P)

    cov_scale = (1.0 - MOMENTUM) / float(n_total)

    with tile.TileContext(nc) as tc:
        with (
            tc.tile_pool(name="const", bufs=1) as constp,
            tc.tile_pool(name="nsstate", bufs=2) as nsp,
            tc.tile_pool(name="ns32", bufs=1) as ns32p,
            tc.tile_pool(name="nstmp", bufs=2) as nstmpp,
            tc.tile_pool(name="p2ps", bufs=2, space="PSUM") as p2ps,
            tc.tile_pool(name="dram", bufs=1, space="DRAM") as dramp,
        ):
            # ---- constants ----
            id16 = constp.tile([P, P], f16, name="id16")
            nc.sync.dma_start(id16[:], id16_d.ap())
            id32 = constp.tile([P, P], f32, name="id32")
            nc.sync.dma_start(id32[:], id32_d.ap())
            e15 = []
            for mi in range(MC):
                t = constp.tile([P, D], f16, name=f"e15_{mi}")
                nc.sync.dma_start(t[:], e15_v[:, mi, :])
                e15.append(t)

            a32 = [
                ns32p.tile([P, D], f32, name=f"a32_{mi}") for mi in range(MC)
            ]

            # ---- phase 1: covariance accumulation ----
            with (
                tc.tile_pool(name="covps", bufs=1, space="PSUM") as covps,
                tc.tile_pool(name="p1x", bufs=3) as p1xp,
                tc.tile_pool(name="mid", bufs=1) as midp,
            ):
                cov_ps = [
                    covps.tile([P, D], f32, name=f"cov{mi}") for mi in range(MC)
                ]
                for c in range(nchunk):
                    xt = p1xp.tile([P, g, D], f16, name="p1chunk", tag="p1chunk")
                    # SWDGE cast-DMA: f32 HBM -> fp16 SBUF
                    nc.gpsimd.dma_start(xt[:], x_v[:, c * g : (c + 1) * g, :])
                    for j in range(g):
                        t = c * g + j
                        for mi in range(MC):
                            nc.tensor.matmul(
                                cov_ps[mi][:],
                                xt[:, j, _cols(mi)],
                                xt[:, j, :],
                                start=(t == 0),
                                stop=(t == tpc - 1),
                            )

                # ---- AllReduce partial covariance (pre-scaled by 0.9/N) ----
                s_stage = midp.tile([P, MC, D], f32, name="s_stage")
                for mi in range(MC):
                    nc.vector.tensor_scalar_mul(
                        s_stage[:, mi, :], cov_ps[mi][:], cov_scale
                    )
                cc_in = dramp.tile([D, D], f32, name="cc_in")
                cc_out = dramp.tile([D, D], f32, name="cc_out", addr_space="Shared")
                cc_in_v = cc_in.rearrange("(t p) m -> p t m", p=P)
                cc_out_v = cc_out.rearrange("(t p) m -> p t m", p=P)
                nc.gpsimd.dma_start(cc_in_v[:, :, :], s_stage[:])
                nc.gpsimd.collective_compute(
                    "AllReduce",
                    mybir.AluOpType.add,
                    replica_groups=[list(range(N_CORES))],
                    ins=[cc_in[:]],
                    outs=[cc_out[:]],
                )
                s_sum = midp.tile([P, MC, D], f32, name="s_sum")
                nc.sync.dma_start(s_sum[:], cc_out_v[:, :, :])

                # ---- C = max(0.9*covar + 0.1*rc, EPS); A = C / C_SCALE ----
                rc_sb = midp.tile([P, MC, D], f32, name="rc_sb")
                nc.sync.dma_start(rc_sb[:], rc_v[:, :, :])
                for mi in range(MC):
                    nc.vector.tensor_scalar_mul(
                        a32[mi][:], rc_sb[:, mi, :], MOMENTUM
                    )
                    nc.vector.tensor_tensor(
                        a32[mi][:], a32[mi][:], s_sum[:, mi, :],
                        mybir.AluOpType.add,
                    )
                    nc.vector.tensor_scalar(
                        a32[mi][:], a32[mi][:], EPS, 1.0 / C_SCALE,
                        mybir.AluOpType.max, mybir.AluOpType.mult,
                    )

            # ---- Newton-Schulz (fp16): Y0 = A, Z0 = I ----
            with tc.tile_pool(name="nsps", bufs=3, space="PSUM") as nsps:
                Y, Z = [], []
                for mi in range(MC):
                    y0 = nsp.tile([P, D], f16, name=f"y0_{mi}", tag=f"Y{mi}")
                    nc.vector.tensor_copy(y0[:], a32[mi][:])
                    Y.append(y0)
                    z0 = nsp.tile([P, D], f16, name=f"z0_{mi}", tag=f"Z{mi}")
                    nc.sync.dma_start(z0[:], e16_v[:, mi, :])
                    Z.append(z0)

                for it in range(ns_iters):
                    T = []
                    for mi in range(MC):
                        pps = nsps.tile([P, D], f32, name="ns_ps", tag="ns_ps")
                        for ki in range(MC):
                            nc.tensor.matmul(
                                pps[:],
                                Z[ki][:, _cols(mi)],
                                Y[ki][:],
                                start=(ki == 0),
                                stop=(ki == MC - 1),
                            )
                        tt = nsp.tile([P, D], f16, name=f"t_{mi}", tag=f"T{mi}")
                        nc.vector.tensor_scalar_mul(tt[:], pps[:], -0.5)
                        nc.vector.tensor_tensor(
                            tt[:], tt[:], e15[mi][:], mybir.AluOpType.add
                        )
                        T.append(tt)
                    newY, newZ = [], []
                    for mi in range(MC):
                        yps = nsps.tile([P, D], f32, name="ns_ps", tag="ns_ps")
                        for ki in range(MC):
                            nc.tensor.matmul(
                                yps[:],
                                Y[ki][:, _cols(mi)],
                                T[ki][:],
                                start=(ki == 0),
                                stop=(ki == MC - 1),
                            )
                        ny = nsp.tile([P, D], f16, name=f"ny_{mi}", tag=f"Y{mi}")
                        nc.vector.tensor_copy(ny[:], yps[:])
                        newY.append(ny)

                        zps = nsps.tile([P, D], f32, name="ns_ps", tag="ns_ps")
                        for ki in range(MC):
                            nc.tensor.matmul(
                                zps[:],
                                T[ki][:, _cols(mi)],
                                Z[ki][:],
                                start=(ki == 0),
                                stop=(ki == MC - 1),
                            )
                        nz = nsp.tile([P, D], f16, name=f"nz_{mi}", tag=f"Z{mi}")
                        nc.vector.tensor_copy(nz[:], zps[:])
                        newZ.append(nz)
                    Y, Z = newY, newZ

                # ---- fp32 polish: X' = 1.5 X - 0.5 X (A X^2); B = X'/sqrt(c)
                X, Xt = [], []
                for mi in range(MC):
                    t = ns32p.tile([P, D], f32, name=f"x32_{mi}")
                    nc.vector.tensor_copy(t[:], Z[mi][:])
                    X.append(t)
                for mi in range(MC):
                    tps = nsps.tile([P, D], f32, name="ns_ps", tag="ns_ps")
                    for ki in range(MC):
                        nc.tensor.transpose(
                            tps[:, _cols(ki)], X[ki][:, _cols(mi)], id32[:]
                        )
                    t = ns32p.tile([P, D], f32, name=f"xt32_{mi}")
                    nc.vector.tensor_copy(t[:], tps[:])
                    Xt.append(t)
                Gm = []
                for mi in range(MC):
                    gps = nsps.tile([P, D], f32, name="ns_ps", tag="ns_ps")
                    for ki in range(MC):
                        nc.tensor.matmul(
                            gps[:], Xt[ki][:, _cols(mi)], X[ki][:],
                            start=(ki == 0), stop=(ki == MC - 1),
                        )
                    t = ns32p.tile([P, D], f32, name=f"g32_{mi}")
                    nc.vector.tensor_copy(t[:], gps[:])
                    Gm.append(t)
                Hm = []
                for mi in range(MC):
                    hps = nsps.tile([P, D], f32, name="ns_ps", tag="ns_ps")
                    for ki in range(MC):
                        nc.tensor.matmul(
                            hps[:], a32[ki][:, _cols(mi)], Gm[ki][:],
                            start=(ki == 0), stop=(ki == MC - 1),
                        )
                    t = ns32p.tile([P, D], f32, name=f"h32_{mi}")
                    nc.vector.tensor_copy(t[:], hps[:])
                    Hm.append(t)
                B16 = []
                for mi in range(MC):
                    wps = nsps.tile([P, D], f32, name="ns_ps", tag="ns_ps")
                    for ki in range(MC):
                        nc.tensor.matmul(
                            wps[:], Xt[ki][:, _cols(mi)], Hm[ki][:],
                            start=(ki == 0), stop=(ki == MC - 1),
                        )
                    tmp = nstmpp.tile([P, D], f32, name="b_tmp", tag="b_tmp")
                    nc.vector.tensor_scalar_mul(
                        tmp[:], X[mi][:], 1.5 * INV_SQRT_C
                    )
                    ws = nstmpp.tile([P, D], f32, name="b_ws", tag="b_ws")
                    nc.vector.tensor_scalar_mul(
                        ws[:], wps[:], -0.5 * INV_SQRT_C
                    )
                    b = constp.tile([P, D], f16, name=f"b16_{mi}")
                    nc.vector.tensor_tensor(
                        b[:], tmp[:], ws[:], mybir.AluOpType.add
                    )
                    B16.append(b)

            # ---- phase 2: whiten  y = x @ B ----
            with (
                tc.tile_pool(name="p2x", bufs=3) as p2xp,
                tc.tile_pool(name="p2xt", bufs=8) as p2xtp,
                tc.tile_pool(name="p2y", bufs=2) as p2yp,
            ):
                for c in range(nchunk):
                    xt = p2xp.tile([P, g, D], f16, name="p2chunk", tag="p2chunk")
                    nc.gpsimd.dma_start(xt[:], x_v[:, c * g : (c + 1) * g, :])
                    ych = p2yp.tile([P, g, D], f32, name="ychunk", tag="ychunk")
                    for j in range(g):
                        tps = p2ps.tile([P, D], f16, name="xt_ps", tag="xt_ps")
                        for ki in range(MC):
                            nc.tensor.transpose(
                                tps[:, _cols(ki)], xt[:, j, _cols(ki)], id16[:]
                            )
                        xts = p2xtp.tile([P, D], f16, name="xt_sb", tag="xt_sb")
                        nc.vector.tensor_copy(xts[:], tps[:])
                        yps = p2ps.tile([P, D], f32, name="y_ps", tag="y_ps")
                        for ki in range(MC):
                            nc.tensor.matmul(
                                yps[:], xts[:, _cols(ki)], B16[ki][:],
                                start=(ki == 0), stop=(ki == MC - 1),
                            )
                        nc.vector.tensor_copy(ych[:, j, :], yps[:])
                    nc.sync.dma_start(y_v[:, c * g : (c + 1) * g, :], ych[:])

    nc.compile()
    return nc


def _const_inputs():
    eye = np.eye(D, dtype=np.float32)
    return {
        "eye15": (1.5 * eye).astype(np.float16),
        "eye16": eye.astype(np.float16),
        "id128_16": np.eye(P, dtype=np.float16),
        "id128_32": np.eye(P, dtype=np.float32),
    }


_PROGRAM_CACHE = {}


def kernel(x, running_covar):
    x = np.ascontiguousarray(np.asarray(x, dtype=np.float32))
    rc = np.ascontiguousarray(np.asarray(running_covar, dtype=np.float32))
    assert x.shape == (N_TOTAL, D) and rc.shape == (D, D)

    if "nc" not in _PROGRAM_CACHE:
        _PROGRAM_CACHE["nc"] = build_program()
    nc = _PROGRAM_CACHE["nc"]

    consts = _const_inputs()
    in_maps = []
    for c in range(N_CORES):
        m = {"x": x[c * SHARD : (c + 1) * SHARD], "running_covar": rc}
        m.update(consts)
        in_maps.append(m)

    res = bass_utils.run_bass_kernel_spmd(
        nc, in_maps, core_ids=list(range(N_CORES))
    )
    return np.concatenate(
        [res.results[c]["y"] for c in range(N_CORES)], axis=0
    )
